# revision 52
# baseline (speedup 1.0000x reference)
"""Trainium2 Bass kernel for nn_Attention_54245436948569.

Full multi-head attention (qkv proj + interleaved RoPE + softmax attention +
out proj) for B=2, N=2048, D=1024, H=16, DH=64, sharded over 8 NeuronCores as
(batch x head-group): core c handles batch c//4 and heads [4*(c%4), 4*(c%4)+4).

Per-core kernel computes a row-parallel partial of the out-projection
([2048, 1024] fp32); the host sums the 4 partials per batch and adds b_out
(the unshard step for row-parallel tensor parallelism).

Matmuls run in bf16 (fp32 PSUM accumulation); softmax runs in fp32 on the
scalar engine with the 1/sqrt(DH) scale folded into exp. The softmax
denominator rides the AV matmul as a ones-column appended to V; the
reciprocal is broadcast across partitions with a K=1 matmul.

Schedule notes:
- DMA submissions are emitted first, in priority order (k/q weights and
  token-block 0 first), across the three DMA-capable engine queues, so the
  first score matmul only waits on ~1.8MB of input.
- The scalar engine's exp stream paces the kernel; each of the 8 (pair,
  q-block) attention blocks runs 16 kt slots of [scores pair -> exp] with
  all other PE work (remaining qkv projections, V projection, out
  projection, normalization broadcasts) woven into the slots.
- AV matmuls run in batches of two kt behind an explicit 4-slot lag: the
  row-tiled score pair occupies both PE weight planes, so the first AV
  after it always pays a ~120ns LDWEIGHTS stall; batching halves the count.
- Each block's last 4 AVs and its normalization spill into the next
  block's first slots; the out-projection for q-block qb runs inside block
  5+qb (qb=3 in the tail).
"""

import numpy as np
import ml_dtypes

B, N, D = 2, 2048, 1024
H, DH = 16, 64
THETA = 10000.0

BF = ml_dtypes.bfloat16

_CACHE = {}


def _build():
    from contextlib import ExitStack
    import concourse.mybir as mybir
    import concourse.tile as tile
    from concourse import bacc
    from concourse.compiler_utils import get_compiler_flags, set_compiler_flags

    set_compiler_flags([f.replace("--enable-ldw-opt=false", "--enable-ldw-opt=true")
                        for f in get_compiler_flags()])

    FP32 = mybir.dt.float32
    F32R = mybir.dt.float32r
    BF16 = mybir.dt.bfloat16
    AF = mybir.ActivationFunctionType
    MUL = mybir.AluOpType.mult
    ADD = mybir.AluOpType.add

    nc = bacc.Bacc(None, target_bir_lowering=False)

    NT = N // 512            # 4 token 512-blocks
    KT_D = D // 128          # 8 contraction tiles for qkv
    KT_N = N // 128          # 16 k-token tiles for attention
    SCALE = 1.0 / float(np.sqrt(DH))
    LAG = 4                  # av trails sc_exp by this many kt slots

    with tile.TileContext(nc) as tc:
        with tc.tile_pool(name="dram", bufs=1, space="DRAM") as dram:
            xT_d = dram.tile([NT, 128, KT_D, 512], BF16, kind="ExternalInput", name="xT", uniquify=False)
            # wqk split into the 4 m-chunks (q pair0/1, k pair0/1) so the
            # lead-in only depends on 512KB of weights, not 1MB.
            wqk_d = dram.tile([4, 128, KT_D, 128], BF16, kind="ExternalInput", name="wqk", uniquify=False)
            wv_d = dram.tile([128, KT_D, 256], BF16, kind="ExternalInput", name="wv", uniquify=False)
            wo_d = dram.tile([128, 2, 1024], BF16, kind="ExternalInput", name="wo", uniquify=False)
            cos_d = dram.tile([NT, 128, 512], BF16, kind="ExternalInput", name="cos2", uniquify=False)
            sin_d = dram.tile([NT, 128, 512], BF16, kind="ExternalInput", name="sin2n", uniquify=False)
            out_d = dram.tile([KT_N, 128, D], BF16, kind="ExternalOutput", name="out", uniquify=False)

            ctx = ExitStack()
            const = ctx.enter_context(tc.tile_pool(name="const", bufs=1))
            ropep = ctx.enter_context(tc.tile_pool(name="ropep", bufs=4))
            attnp = ctx.enter_context(tc.tile_pool(name="attnp", bufs=8))
            stkp = ctx.enter_context(tc.tile_pool(name="stkp", bufs=8))
            normp = ctx.enter_context(tc.tile_pool(name="normp", bufs=3))
            outp = ctx.enter_context(tc.tile_pool(name="outp", bufs=3))
            # PSUM budget (8 banks): misc 2x1, scores 2x2, av 2x1
            ps_misc = ctx.enter_context(tc.tile_pool(name="ps_misc", bufs=2, space="PSUM"))
            ps_sc = ctx.enter_context(tc.tile_pool(name="ps_sc", bufs=2, space="PSUM"))
            ps_av = ctx.enter_context(tc.tile_pool(name="ps_av", bufs=2, space="PSUM"))

            # ---- persistent SBUF tensors ----
            wqk_m = [const.tile([128, KT_D, 128], BF16, name=f"wqk_m{m}") for m in range(4)]
            wv = const.tile([128, KT_D, 256], BF16)
            wo = const.tile([128, 2, 1024], BF16)
            cos_nt = [const.tile([128, 512], BF16, name=f"cos_{nt}") for nt in range(NT)]
            sin_nt = [const.tile([128, 512], BF16, name=f"sin_{nt}") for nt in range(NT)]
            q2n = [[const.tile([128, 512], BF16, name=f"q2_{p}_{nt}") for nt in range(NT)] for p in range(2)]
            k2n = [[const.tile([128, 512], BF16, name=f"k2_{p}_{nt}") for nt in range(NT)] for p in range(2)]
            v_t = [const.tile([128, 4, 65], BF16, name=f"v_{tt}") for tt in range(KT_N)]
            ones1 = const.tile([128, 64], BF16)
            ones1f = const.tile([128, 64], FP32)
            xT_nt = [const.tile([128, KT_D, 512], BF16, name=f"xT_{nt}") for nt in range(NT)]

            with nc.named_scope("load"):
                # ALL input DMA on one engine, in exact need-order: a single
                # dma_start spreads its descriptors across all 16 hardware
                # queues, so one engine saturates DMA bandwidth while giving
                # strict global FIFO priority (three engines racing dilutes
                # the high-priority transfers).
                nc.sync.dma_start(out=wqk_m[2][:], in_=wqk_d[2])
                nc.sync.dma_start(out=xT_nt[0][:], in_=xT_d[0])
                nc.sync.dma_start(out=wqk_m[0][:], in_=wqk_d[0])
                nc.sync.dma_start(out=cos_nt[0][:], in_=cos_d[0])
                nc.sync.dma_start(out=sin_nt[0][:], in_=sin_d[0])
                nc.sync.dma_start(out=wv[:], in_=wv_d[:])
                for nt in range(1, NT):
                    # trig before its xT chunk: the rope consumes cos/sin
                    # immediately after the projection matmuls finish
                    nc.sync.dma_start(out=cos_nt[nt][:], in_=cos_d[nt])
                    nc.sync.dma_start(out=sin_nt[nt][:], in_=sin_d[nt])
                    nc.sync.dma_start(out=xT_nt[nt][:], in_=xT_d[nt])
                nc.sync.dma_start(out=wqk_m[3][:], in_=wqk_d[3])
                nc.sync.dma_start(out=wqk_m[1][:], in_=wqk_d[1])
                nc.sync.dma_start(out=wo[:], in_=wo_d[:])

                # PE warmup under the DMA wait: junk matmuls (no input deps)
                # so the HAM clock gate is released before real work.
                wsrc = const.tile([128, 512], BF16)
                nc.vector.memset(wsrc[:], 0.5)
                pw = ps_misc.tile([128, 512], FP32, tag="misc", name="pw")
                for r in range(14):
                    nc.tensor.matmul(pw[:], wsrc[:, 0:128], wsrc[:],
                                     start=(r == 0), stop=(r == 13))
                nc.vector.memset(ones1[:], 1.0)
                nc.vector.memset(ones1f[:], 1.0)
                # preload the exp table set while DMAs are in flight
                warmup = const.tile([128, 8], FP32)
                nc.scalar.activation(warmup[:], ones1[:, 0:8], AF.Exp, scale=0.125)
                for tt in range(KT_N):
                    nc.vector.memset(v_t[tt][:, :, 64:65], 1.0)

            pair_mask = []
            for i in range(16):
                pair_mask += [2 * i + 1, 2 * i]

            # ---- building blocks ----
            def qk_mms(m, nt, pqk, kts):
                for kt in kts:
                    nc.tensor.matmul(
                        pqk[:],
                        wqk_m[m][:, kt, :],
                        xT_nt[nt][:, kt, :],
                        start=(kt == 0), stop=(kt == KT_D - 1),
                    )

            def qk_rope(m, nt, pqk, evict_engine, alu=None):
                dest = (q2n if m < 2 else k2n)[m % 2][nt]
                qraw = ropep.tile([128, 512], BF16, name="qraw")
                if evict_engine == "scalar":
                    nc.scalar.activation(qraw[:], pqk[:], AF.Copy)
                else:
                    nc.vector.tensor_copy(qraw[:], pqk[:])
                qcos = ropep.tile([128, 512], BF16, name="qcos")
                qsw = ropep.tile([128, 512], BF16, name="qsw")
                tmp = ropep.tile([128, 512], BF16, name="tmp")
                # in-block: SBUF-only ALU rides the otherwise-idle gpsimd
                # engine; lead-in: vector (gpsimd is ~2x slower per op and
                # the lead-in rope chain is latency-critical).
                alu = alu or nc.gpsimd
                alu.tensor_tensor(out=qcos[:], in0=qraw[:], in1=cos_nt[nt][:], op=MUL)
                nc.vector.stream_shuffle(qsw[:], qraw[:], pair_mask)
                alu.tensor_tensor(out=tmp[:], in0=qsw[:], in1=sin_nt[nt][:], op=MUL)
                alu.tensor_tensor(out=dest[:], in0=qcos[:], in1=tmp[:], op=ADD)

            def qk_proj_nt(m, nt, evict_engine, alu=None):
                pqk = ps_misc.tile([128, 512], FP32, tag="misc", name="pqk")
                qk_mms(m, nt, pqk, range(KT_D))
                qk_rope(m, nt, pqk, evict_engine, alu)

            def v_proj(tt):
                pv = ps_misc.tile([128, 512], FP32, tag="misc", name="pv")
                for kt in range(KT_D):
                    nc.tensor.matmul(
                        pv[:, 0:256],
                        xT_nt[tt // 4][:, kt, (tt % 4) * 128:(tt % 4 + 1) * 128],
                        wv[:, kt, :],
                        start=(kt == 0), stop=(kt == KT_D - 1),
                    )
                nc.vector.tensor_copy(v_t[tt][:, :, 0:64], pv[:, 0:256].rearrange("p (h d) -> p h d", d=64))

            # per-block attention state
            class Blk:
                def __init__(self, p, qb):
                    self.p, self.qb = p, qb
                    self.attnT = {}

            def sc_exp(st, kt):
                p, qb = st.p, st.qb
                with nc.named_scope(f"scores_p{p}_qb{qb}"):
                    pg = ps_sc.tile([128, 2, 512], FP32, tag="pg", name="pg")
                    attnT = attnp.tile([128, 2, 512], BF16, tag="attnT", name="attnT")
                    st.attnT[kt] = attnT
                    knt, ko = kt // 4, (kt % 4) * 128
                    # NOTE: a 2x2 row+col xbus pack (4 K=64/M=64 tiles) was
                    # measured NET SLOWER: it trips the P0 power downclock
                    # (all MMs 2.4->2.0GHz, +20% everywhere). Keep row pair.
                    nc.tensor.matmul(
                        pg[:, 0, :], k2n[p][knt][0:64, ko:ko + 128], q2n[p][qb][0:64, :],
                        start=True, stop=True, tile_position=(0, 0),
                    )
                    nc.tensor.matmul(
                        pg[:, 1, :], k2n[p][knt][64:128, ko:ko + 128], q2n[p][qb][64:128, :],
                        start=True, stop=True, tile_position=(64, 0),
                    )
                    nc.scalar.activation(attnT[:], pg[:], AF.Exp, scale=SCALE)

            def av(st, kt):
                p, qb = st.p, st.qb
                with nc.named_scope(f"scores_p{p}_qb{qb}"):
                    if kt == 0:
                        st.pav_a = ps_av.tile([128, 512], FP32, tag="pav", name="pav_a")
                        st.pav_b = ps_av.tile([128, 512], FP32, tag="pav", name="pav_b")
                    attnT = st.attnT.pop(kt)
                    nc.tensor.matmul(
                        st.pav_a[0:65, :], v_t[kt][:, 2 * p, :], attnT[:, 0, :],
                        start=(kt == 0), stop=(kt == KT_N - 1),
                    )
                    nc.tensor.matmul(
                        st.pav_b[0:65, :], v_t[kt][:, 2 * p + 1, :], attnT[:, 1, :],
                        start=(kt == 0), stop=(kt == KT_N - 1),
                    )

            def norm_a(st):
                # copy AV accumulators (numerator rows 0:64 + the ones-ride
                # denominator row 64) off PSUM; frees the pav bank pair.
                with nc.named_scope(f"norm_p{st.p}_qb{st.qb}"):
                    # bf16 is enough precision here (denominator row scale
                    # error ~0.4% rms, well inside the 2e-2 budget) and the
                    # bf16 broadcast matmul streams at full rate vs the slow
                    # F32R path (~436ns isolated)
                    st.ua = stkp.tile([128, 512], BF16, name="ua", bufs=2)
                    st.ub = stkp.tile([128, 512], BF16, name="ub", bufs=2)
                    nc.vector.tensor_copy(st.ua[0:65, :], st.pav_a[0:65, :])
                    nc.vector.tensor_copy(st.ub[0:65, :], st.pav_b[0:65, :])

            def norm_b(st):
                # broadcast each denominator row across 64 partitions with a
                # K=1 matmul, adjacent to streaming MMs so it pipelines.
                with nc.named_scope(f"norm_p{st.p}_qb{st.qb}"):
                    st.pbc = ps_misc.tile([128, 512], FP32, tag="misc", name="pbc")
                    st.pbc2 = ps_misc.tile([128, 512], FP32, tag="misc", name="pbc2")
                    nc.tensor.matmul(
                        st.pbc[0:64, :], ones1[64:65, :],
                        st.ua[64:65, :],
                        start=True, stop=True, tile_position=(64, 0),
                    )
                    nc.tensor.matmul(
                        st.pbc2[0:64, :], ones1[64:65, :],
                        st.ub[64:65, :],
                        start=True, stop=True, tile_position=(64, 0),
                    )

            def norm_c(st):
                # per-qs slices (separate [128,128] tiles): each
                # out-projection slice depends only on its own slice's
                # recip/mult/shift chain, not the full-width tile (the
                # whole-tile dependency cost ~0.9us at every oproj start).
                with nc.named_scope(f"norm_p{st.p}_qb{st.qb}"):
                    sts = []
                    for qs in range(4):
                        cs = slice(qs * 128, (qs + 1) * 128)
                        sq = stkp.tile([128, 128], BF16, name=f"stknq{qs}")
                        tq = stkp.tile([128, 128], BF16, name=f"tmpnq{qs}", bufs=2)
                        ra = normp.tile([128, 128], FP32, name=f"recq{qs}", bufs=2)
                        rb = normp.tile([128, 128], FP32, name=f"recq2_{qs}", bufs=2)
                        nc.vector.reciprocal_approx_fast(out=ra[0:64, :], in_=st.pbc[0:64, cs])
                        nc.vector.reciprocal_approx_fast(out=rb[0:64, :], in_=st.pbc2[0:64, cs])
                        nc.gpsimd.tensor_tensor(out=sq[0:64, :], in0=st.ua[0:64, cs], in1=ra[0:64, :], op=MUL)
                        nc.gpsimd.tensor_tensor(out=tq[0:64, :], in0=st.ub[0:64, cs], in1=rb[0:64, :], op=MUL)
                        nc.sync.dma_start(out=sq[64:128, :], in_=tq[0:64, :])
                        sts.append(sq)
                    stknv[(st.p, st.qb)] = sts

            stknv = {}   # (p, qb) -> stkn tile
            ostates = {}

            def oproj_piece(qb, qs, dt, evict):
                with nc.named_scope(f"oproj_qb{qb}"):
                    if dt == 0:
                        ostates[(qb, qs)] = outp.tile([128, 1024], BF16, name="ostg")
                    ostg = ostates[(qb, qs)]
                    po = ps_misc.tile([128, 512], FP32, tag="misc", name="po")
                    for p in range(2):
                        nc.tensor.matmul(
                            po[:],
                            stknv[(p, qb)][qs][:],
                            wo[:, p, dt * 512:(dt + 1) * 512],
                            start=(p == 0), stop=(p == 1),
                        )
                    if evict == "scalar":
                        nc.scalar.activation(ostg[:, dt * 512:(dt + 1) * 512], po[:], AF.Copy)
                    else:
                        nc.vector.tensor_copy(ostg[:, dt * 512:(dt + 1) * 512], po[:])
                    # per-half out DMA so the final transfer granule is 128KB
                    eng = (nc.sync, nc.scalar, nc.gpsimd)[(2 * qs + dt) % 3]
                    eng.dma_start(out=out_d[qb * 4 + qs, :, dt * 512:(dt + 1) * 512],
                                  in_=ostg[:, dt * 512:(dt + 1) * 512])

            # ---- lead-in compute ----
            with nc.named_scope("qkv"):
                qk_proj_nt(2, 0, "scalar", alu=nc.vector)   # k pair0 nt0
                qk_proj_nt(0, 0, "scalar", alu=nc.vector)   # q pair0 nt0

            # ---- extras: (block, slot) -> closures ----
            extras = [dict() for _ in range(8)]

            def add(bi, slot, fn):
                extras[bi].setdefault(slot, []).append(fn)

            def proj_bundle(m, nt, ev="vector", alu=None):
                return lambda: qk_proj_nt(m, nt, ev, alu)

            # block 0 (DMA-paced): v tiles + remaining pair0 projections,
            # placed to match DMA arrival order (wv, xT1, xT2, xT3). The
            # same-block-consumed k projections go in pre_extras (before the
            # slot's sc, which reads their output).
            # these three gate the exp stream directly (sc(4nt) reads them
            # right after the xT chunk lands) -> low-latency vector ALU rope
            pre_extras = {
                4: [proj_bundle(2, 1, alu=nc.vector)],   # k pair0 nt1 (xT1)
                8: [proj_bundle(2, 2, alu=nc.vector)],   # k pair0 nt2 (xT2)
                12: [proj_bundle(2, 3, alu=nc.vector)],  # k pair0 nt3 (xT3)
            }
            add(0, 2, lambda: v_proj(0))
            add(0, 3, lambda: v_proj(1))
            for s, tts in ((5, (2, 3)), (6, (4, 5)), (7, (6,))):
                for tt in tts:
                    add(0, s, lambda tt=tt: v_proj(tt))
            for s, tts in ((9, (7, 8)), (10, (9, 10)), (11, (11,))):
                for tt in tts:
                    add(0, s, lambda tt=tt: v_proj(tt))
            add(0, 13, lambda: v_proj(12))
            add(0, 14, proj_bundle(0, 1))         # q pair0 nt1 (needed b1 s0)
            add(0, 14, lambda: v_proj(13))
            add(0, 15, lambda: v_proj(14))
            add(0, 15, lambda: v_proj(15))
            # blocks 1-5: remaining projections (deadlines in comments)
            sched = [
                (1, 0, 0, 2),   # q pair0 nt2   (b2 s0)
                (1, 6, 3, 0),   # k pair1 nt0   (b4 s0)
                (1, 12, 0, 3),  # q pair0 nt3   (b3 s0)
                (2, 0, 3, 1),   # k pair1 nt1   (b4 s4)
                (2, 6, 3, 2),   # k pair1 nt2   (b4 s8)
                (2, 12, 1, 0),  # q pair1 qb0   (b4 s0)
                (3, 0, 3, 3),   # k pair1 nt3   (b4 s12)
                (3, 6, 1, 1),   # q pair1 qb1   (b5 s0)
                (4, 0, 1, 2),   # q pair1 qb2   (b6 s0)
                (4, 8, 1, 3),   # q pair1 qb3   (b7 s0)
            ]
            for bi, s, m, nt in sched:
                add(bi, s, proj_bundle(m, nt))
            # blocks 5-7: out-projection for qb 0,1,2 (qb3 in the tail)
            for bi, qb in zip((5, 6, 7), (0, 1, 2)):
                for j, (qs, dt) in enumerate((q, d) for q in range(4) for d in range(2)):
                    add(bi, 6 + j, lambda qb=qb, qs=qs, dt=dt: oproj_piece(qb, qs, dt, "vector"))

            # av emission slots: batched pairs at odd slots (LAG=4); the last
            # block ramps down in-block instead of spilling.
            AV_SLOTS = {5: (0, 1), 7: (2, 3), 9: (4, 5), 11: (6, 7), 13: (8, 9), 15: (10, 11)}
            AV_SLOTS_LAST = {5: (0, 1), 7: (2, 3), 9: (4, 5), 11: (6, 7),
                             12: (8, 9), 13: (10, 11), 14: (12, 13), 15: (14, 15)}

            # ---- main emission: 8 blocks x 16 slots ----
            order = [(0, qb) for qb in range(NT)] + [(1, qb) for qb in range(NT)]
            blks = [Blk(p, qb) for (p, qb) in order]
            for bi in range(8):
                st = blks[bi]
                prev = blks[bi - 1] if bi > 0 else None
                av_slots = AV_SLOTS_LAST if bi == 7 else AV_SLOTS
                for kt in range(KT_N):
                    if bi == 0:
                        for fn in pre_extras.get(kt, ()):
                            fn()
                    sc_exp(st, kt)
                    if prev is not None:
                        if kt == 1:
                            av(prev, 12)
                            av(prev, 13)
                        elif kt == 3:
                            av(prev, 14)
                            av(prev, 15)
                            norm_a(prev)
                        elif kt == 5:
                            # one slot of slack after norm_a's vector copies:
                            # at kt4 the K=1 broadcast MMs ran at isolated
                            # latency (~436ns + PE gap) waiting on them
                            norm_b(prev)
                        elif kt == 6:
                            norm_c(prev)
                    for fn in extras[bi].get(kt, ()):
                        fn()
                    for akt in av_slots.get(kt, ()):
                        av(st, akt)

            # ---- tail: last block's norm + final out-projection ----
            # norm_c is per-qs, so each oproj slice launches as soon as its
            # slice is normalized; evictions alternate scalar/vector (8
            # serialized scalar copies were the tail's critical path).
            st = blks[7]
            norm_a(st)
            norm_b(st)
            norm_c(st)
            for qs in range(4):
                oproj_piece(3, qs, 0, "vector" if qs % 2 else "scalar")
                oproj_piece(3, qs, 1, "scalar" if qs % 2 else "vector")

            ctx.close()

    nc.compile()
    return nc


def _host_prep(hidden_states, w_qkv):
    """Per-core input maps (host-side shard + layout prep)."""
    invf = 1.0 / (THETA ** (np.arange(0, DH, 2, dtype=np.float32) / DH))
    t = np.arange(N, dtype=np.float32)
    d_idx = np.arange(128)
    f = invf[(d_idx % 64) // 2]
    ang = t[None, :] * f[:, None]
    cos2 = np.ascontiguousarray(np.cos(ang)).astype(BF)
    sign = np.where(d_idx % 2 == 0, -1.0, 1.0).astype(np.float32)
    sin2n = np.ascontiguousarray(np.sin(ang) * sign[:, None]).astype(BF)
    # [NT, 128, 512] chunks
    cos_c = np.ascontiguousarray(cos2.reshape(128, NT_, 512).transpose(1, 0, 2))
    sin_c = np.ascontiguousarray(sin2n.reshape(128, NT_, 512).transpose(1, 0, 2))

    # [NT, 128, KT_D, 512] partition-major so device DMAs are contiguous
    xT_b = [np.ascontiguousarray(
                hidden_states[b].T.astype(BF).reshape(D // 128, 128, N // 512, 512)
                .transpose(2, 1, 0, 3))
            for b in range(B)]

    in_maps = []
    for c in range(8):
        b, g = c // 4, c % 4
        heads = [4 * g, 4 * g + 1, 4 * g + 2, 4 * g + 3]
        cols = []
        for off in (0, 1024):
            for h in heads:
                cols.append(w_qkv[:, off + h * 64: off + (h + 1) * 64])
        # [4, 128, KT_D, 128]: m-chunk major (q pair0, q pair1, k pair0, k pair1)
        wqk = (np.concatenate(cols, axis=1).astype(BF)
               .reshape(D // 128, 128, 4, 128).transpose(2, 1, 0, 3))
        wv = np.ascontiguousarray(
            np.concatenate([w_qkv[:, 2048 + h * 64: 2048 + (h + 1) * 64] for h in heads],
                           axis=1).astype(BF).reshape(D // 128, 128, 256).transpose(1, 0, 2))
        in_maps.append({
            "xT": xT_b[b],
            "wqk": np.ascontiguousarray(wqk),
            "wv": wv,
            "cos2": cos_c,
            "sin2n": sin_c,
        })
    return in_maps


NT_ = N // 512


def kernel(hidden_states, w_qkv, w_out, b_out, _trace=False, _tmpdir=None):
    hidden_states = np.asarray(hidden_states, dtype=np.float32)
    w_qkv = np.asarray(w_qkv, dtype=np.float32)
    w_out = np.asarray(w_out, dtype=np.float32)
    b_out = np.asarray(b_out, dtype=np.float32)

    from concourse.bass_utils import run_bass_kernel_spmd

    if "nc" not in _CACHE:
        _CACHE["nc"] = _build()
    nc = _CACHE["nc"]

    in_maps = _host_prep(hidden_states, w_qkv)
    for c in range(8):
        g = c % 4
        wo = np.ascontiguousarray(
            w_out[4 * g * 64: 4 * g * 64 + 256, :].astype(BF).reshape(2, 128, 1024).transpose(1, 0, 2))
        in_maps[c]["wo"] = wo

    kwargs = {}
    if _trace:
        kwargs = dict(trace=True, tmpdir=_tmpdir)
    res = run_bass_kernel_spmd(nc, in_maps, core_ids=list(range(8)), **kwargs)

    out = np.zeros((B, N, D), dtype=np.float32)
    for c in range(8):
        out[c // 4] += res.results[c]["out"].reshape(N, D).astype(np.float32)
    out += b_out[None, None, :]
    if _trace:
        _CACHE["last_res"] = res
    return out
